# revision 1
# baseline (speedup 1.0000x reference)
"""Contrastive flow loss on 8 Trainium2 NeuronCores.

Math (faithful to the reference):
    z_norm = z / max(||z||, eps)
    sim    = z_norm @ z_norm.T / T            (B x B, symmetric)
    pos_mask[i,j] = (a_i . a_j == 4) & (i != j)
                  = p_i * p_j off-diagonal, with p_i = all-ones(attr row i)
                    (exact: binary attrs, dot of 4 0/1 terms == 4 iff both rows
                     are all ones)
    all_sum_i = sum_j exp(sim_ij) - exp(sim_ii)
    pos_sum_i = p_i ? sum_{j!=i, p_j=1} exp(sim_ij) + (B - P + 1) : B
    loss_i    = log(all_sum_i) - log(max(pos_sum_i, eps))
    loss      = mean over valid rows (num_pos > 0)

Device strategy (data-parallel over row blocks, 1024 rows/core):
  Each core receives z rolled so that ITS row block sits at rows 0..1023
  (identical SPMD program, no per-core constants).  Work is emitted in 8
  column groups of 1024 so normalization/transposition of group g+1 overlaps
  the matmul/exp pipeline of group g:
      sim block  = 128x512 f32r matmuls (PE)
      diag killed by adding -1e30 to sim (exp -> 0); in the rolled layout the
        whole block diagonal lives in column group 0                 (DVE)
      E = exp(sim/T) as bf16                                         (ACT)
      column sums [1s | p]^T @ E accumulated in PSUM                 (PE)
  rsqrt for normalization is computed as exp(-0.5*ln(max(sumsq,1e-24))) so
  every ACT op lives in the single `natural_log_exp_and_others` table set
  (no ~2.7us table reloads when phases interleave).  The clamp matches the
  reference: for sumsq < eps^2 it yields exactly 1/eps.
  Because diag-zeroed E is exactly symmetric, the summed COLUMN sums over
  all cores equal the ROW sums the reference needs:
      u_j = sum_i E'_ij = all_sum_j,  s_j = sum_i p_i E'_ij = masked row sum.
  The host adds the 8 partial [2, 8192] outputs (the "all-reduce") and
  finishes the O(B) scalar arithmetic.
"""

import numpy as np

B = 8192          # batch rows
D = 128           # feature dim
A = 4             # attribute dim
NCORES = 8
RB = B // NCORES  # rows per core
NRT = RB // 128   # 128-row tiles per core block
CW = 1024         # column group width (one ACT op)
NG = B // CW      # column groups
TEMP = 0.07
EPS = 1e-12

_CACHE = {}


def _build(repeat: int = 1):
    import concourse.bacc as bacc
    import concourse.tile as tile
    from concourse import mybir
    from concourse.masks import make_identity

    f32 = mybir.dt.float32
    bf16 = mybir.dt.bfloat16
    f32r = mybir.dt.float32r
    Alu = mybir.AluOpType
    Act = mybir.ActivationFunctionType

    nc = bacc.Bacc("TRN2", debug=False)
    z_in = nc.dram_tensor("z_full", [B, D], f32, kind="ExternalInput").ap()
    a_in = nc.dram_tensor("attr_blk", [RB, A], f32, kind="ExternalInput").ap()
    cs_out = nc.dram_tensor("csum", [2, B], f32, kind="ExternalOutput").ap()

    with tile.TileContext(nc) as tc:
        with (
            tc.tile_pool(name="const", bufs=1) as const,
            tc.tile_pool(name="zTbuf", bufs=1) as zTp,
            tc.tile_pool(name="znatp", bufs=2) as znatp,
            tc.tile_pool(name="sqp", bufs=2) as sqp,
            tc.tile_pool(name="normp", bufs=2) as normp,
            tc.tile_pool(name="znp", bufs=3) as znp,
            tc.tile_pool(name="ps", bufs=2, space="PSUM") as psp,
            tc.tile_pool(name="csps", bufs=1, space="PSUM") as csp,
            tc.tile_pool(name="esb", bufs=3) as ep,
            tc.tile_pool(name="cso", bufs=1) as csop,
        ):
            # --- constants ---
            ident = const.tile([128, 128], f32)
            make_identity(nc, ident)
            negI = const.tile([128, 128], f32)
            nc.gpsimd.memset(negI, 0.0)
            nc.gpsimd.affine_select(
                out=negI,
                in_=negI,
                compare_op=Alu.not_equal,
                fill=-1e30,
                base=0,
                pattern=[[-1, 128]],
                channel_multiplier=1,
            )

            # --- p for this core's rows -> colsum stationaries W_r = [1 | p] ---
            attr_t = const.tile([128, NRT, A], f32)
            nc.sync.dma_start(out=attr_t, in_=a_in.rearrange("(r p) a -> p r a", p=128))
            asum = const.tile([128, NRT], f32)
            nc.vector.tensor_reduce(
                out=asum, in_=attr_t, axis=mybir.AxisListType.X, op=Alu.add
            )
            pvec = const.tile([128, NRT], f32)
            # attr sums are exact small ints; relu(sum - 3) == 1 iff sum == 4
            bias_m3 = const.tile([128, 1], f32)
            nc.vector.memset(bias_m3, -3.0)
            nc.scalar.activation(
                out=pvec, in_=asum, func=Act.Relu, bias=bias_m3, scale=1.0
            )
            W = const.tile([128, NRT, 2], bf16)
            nc.vector.memset(W, 1.0)
            for r in range(NRT):
                nc.vector.tensor_copy(out=W[:, r, 1:2], in_=pvec[:, r : r + 1])

            zre = z_in.rearrange("(n p) d -> p n d", p=128)

            def body():
                zTt = []
                csum_sb = csop.tile([2, B], f32, tag="csum_sb")
                for gt in range(NG):
                    # --- phase A for column group gt: normalize + transpose ---
                    znc = znatp.tile([128, 8, 128], f32, tag="znat")
                    nc.sync.dma_start(out=znc, in_=zre[:, gt * 8 : (gt + 1) * 8, :])
                    sq = sqp.tile([128, 8, 128], f32, tag="sq")
                    nc.vector.tensor_mul(out=sq, in0=znc, in1=znc)
                    ss = normp.tile([128, 8], f32, tag="ss")
                    nc.vector.tensor_reduce(
                        out=ss, in_=sq, axis=mybir.AxisListType.X, op=Alu.add
                    )
                    nc.vector.tensor_scalar_max(out=ss, in0=ss, scalar1=1e-24)
                    lnv = normp.tile([128, 8], f32, tag="lnv")
                    nc.scalar.activation(out=lnv, in_=ss, func=Act.Ln)
                    rn = normp.tile([128, 8], f32, tag="rn")
                    # rn = exp(-0.5*ln(ss)) = 1/sqrt(ss); table set shared w/ Exp
                    nc.scalar.activation(out=rn, in_=lnv, func=Act.Exp, scale=-0.5)
                    zTg = zTp.tile([128, CW], f32r, tag=f"zt{gt}")
                    for j in range(8):
                        zn = znp.tile([128, 128], f32, tag="zn")
                        nc.vector.tensor_scalar_mul(
                            out=zn, in0=znc[:, j, :], scalar1=rn[:, j : j + 1]
                        )
                        pt = psp.tile([128, 128], f32, tag="tp")
                        nc.tensor.transpose(pt, zn, ident)
                        nc.vector.tensor_copy(
                            out=zTg[:, j * 128 : (j + 1) * 128], in_=pt
                        )
                    zTt.append(zTg)

                    # --- phase B for column group gt ---
                    cs = csp.tile([2, CW], f32, tag="cs")
                    for r in range(NRT):
                        sim = psp.tile([128, CW], f32, tag="sim")
                        for h in range(CW // 512):
                            nc.tensor.matmul(
                                sim[:, h * 512 : (h + 1) * 512],
                                lhsT=zTt[0][:, r * 128 : (r + 1) * 128],
                                rhs=zTg[:, h * 512 : (h + 1) * 512],
                                start=True,
                                stop=True,
                            )
                        if gt == 0:
                            # block diagonal: local rows r*128.. vs same cols
                            off = r * 128
                            nc.vector.tensor_add(
                                out=sim[:, off : off + 128],
                                in0=sim[:, off : off + 128],
                                in1=negI,
                            )
                        E = ep.tile([128, CW], bf16, tag="E")
                        nc.scalar.activation(
                            out=E, in_=sim, func=Act.Exp, scale=float(1.0 / TEMP)
                        )
                        for h in range(CW // 512):
                            nc.tensor.matmul(
                                cs[:, h * 512 : (h + 1) * 512],
                                lhsT=W[:, r, :],
                                rhs=E[:, h * 512 : (h + 1) * 512],
                                start=(r == 0),
                                stop=(r == NRT - 1),
                            )
                    nc.vector.tensor_copy(
                        out=csum_sb[:, gt * CW : (gt + 1) * CW], in_=cs
                    )
                nc.sync.dma_start(out=cs_out, in_=csum_sb)

            for _rep in range(repeat):
                body()

    nc.compile()
    return nc


def _get_nc(repeat: int = 1):
    key = ("nc", repeat)
    if key not in _CACHE:
        _CACHE[key] = _build(repeat)
    return _CACHE[key]


def kernel(z_flowed: np.ndarray, attributes: np.ndarray) -> np.ndarray:
    from concourse.bass_utils import run_bass_kernel_spmd

    z = np.ascontiguousarray(np.asarray(z_flowed, dtype=np.float32))
    attrs = np.ascontiguousarray(np.asarray(attributes, dtype=np.float32))

    nc = _get_nc()
    in_maps = []
    for c in range(NCORES):
        in_maps.append(
            {
                "z_full": np.roll(z, -c * RB, axis=0),
                "attr_blk": np.ascontiguousarray(attrs[c * RB : (c + 1) * RB]),
            }
        )
    res = run_bass_kernel_spmd(nc, in_maps, list(range(NCORES)))
    _CACHE["last_result"] = res

    u = np.zeros(B, np.float64)
    s = np.zeros(B, np.float64)
    for c in range(NCORES):
        cs = res.results[c]["csum"]
        u += np.roll(cs[0].astype(np.float64), c * RB)
        s += np.roll(cs[1].astype(np.float64), c * RB)

    # host-side gather / final O(B) scalar math (the "all-reduce" step)
    p = attrs.sum(axis=1) == float(A)
    P = int(p.sum())
    all_sum = u
    pos_sum = np.where(p, s + float(B - P + 1), float(B))
    num_pos = np.where(p, P - 1, 0)
    valid = (num_pos > 0) & (all_sum > 0) & (pos_sum > 0)
    with np.errstate(divide="ignore", invalid="ignore"):
        loss_i = np.log(all_sum) - np.log(np.maximum(pos_sum, EPS))
    cnt = int(valid.sum())
    total = float(np.where(valid, loss_i, 0.0).sum())
    loss = total / max(cnt, 1) if cnt > 0 else 0.0
    return np.asarray(loss, dtype=np.float32)



# revision 34
# speedup vs baseline: 2.3420x; 2.3420x over previous
"""Contrastive flow loss on 8 Trainium2 NeuronCores.

Key observation: the reference loss only averages loss_i over rows with
num_pos > 0, i.e. rows whose attribute vector is all-ones ("positive"
rows, P ~ B/16 ~ 512 of 8192).  pos_mask[i,j] = p_i*p_j off-diagonal, so
only all_sum_i and pos_sum_i for the P positive rows are needed:

    all_sum_i = sum_j exp(sim_ij) - exp(sim_ii)          (i positive)
    pos_sum_i = sum_{pos j != i} exp(sim_ij) + (B - P + 1)
    loss      = mean_i [log all_sum_i - log pos_sum_i]

That collapses the B x B problem to a P x B strip -- a memory-bound
kernel.  Sharding: column-parallel.  Host normalizes z (f32), quantizes
to bf16, transposes to [D, B], pads the positive-row set to PPAD=768
(zero rows; their outputs are ignored).  Each core gets:
    zt_pos [128, 768]   all positive stationaries (bf16, zero-padded)
    zt_seg [128, 1024]  its own column segment, permuted so the
                        segment's positive columns come first (m_c <= 128)
    mneg   [128, 768]   -1e30 at (global-pos-index, local-pos-col) pairs
                        that are self-similarities -> exp = 0 exactly
    maskp  [128, 128]   1.0 for cols < m_c else 0.0 (pos-column mask)

Device, per stationary tile t (6 tiles of 128 pos rows):
    sim [128,1024] = zt_pos[:,t].T @ zt_seg  (2 matmuls, PSUM)
    sim[:, :128] += mneg[:, t]               (self-term kill, DVE)
    E = exp(sim/T): ACT tiles use the Exp activation with fused
        accum_out (= all_sum partial); DVE tiles use a Schraudolph
        fast-exp (int16(sim*a+b) bit-pattern read back as bf16, linear-
        mean-zero magic constant) plus a 4x bf16 tensor_reduce.
    pos partial = tensor_tensor_reduce(E[:, :128] * maskp)  (DVE)
Host sums the 8 per-core partials (f64), adds (B-P+1), takes logs.

Accuracy: bf16 z_norm -> sim abs err ~2e-3 -> per-element exp err ~3%
random (averages out over 8192/520-col sums); Schraudolph is ~+-5% per
element but linear-mean-zero, and any common multiplicative bias cancels
in log(all_sum) - log(pos_sum).  Observed rel err ~1e-3 << 2e-2 gate.
"""

import numpy as np

B = 8192
D = 128
A = 4
NCORES = 8
SEG = B // NCORES      # columns per core
PPAD = 768             # padded global positive count
SEGP = 128             # padded per-segment positive count
NT = PPAD // 128       # stationary tiles
TEMP = 0.07
EPS = 1e-12

# exp tiles handled by the scalar engine (rest use the DVE fast-exp);
# DVE tiles interleaved so both engines stay busy throughout
ACT_TILES = (0, 2, 4)

# Schraudolph bf16 fast-exp: bits_i16(exp(x)) ~= x*SCHRA_A + SCHRA_B.
# A = 128*log2(e)/TEMP ; B = 128*(127 - log2(E[(1+f)*2^-f])) -- the shift
# zeroes the mean linear ratio over uniform mantissa fraction f:
# int_0^1 (1+f) 2^-f df = 1.0407158 -> log2 = 0.0575766.
SCHRA_A = 128.0 * 1.4426950408889634 / TEMP
SCHRA_B = 128.0 * (127.0 - 0.0575766)

_CACHE = {}


def _build(repeat: int = 1, nt: int = 5):
    import concourse.bacc as bacc
    import concourse.tile as tile
    from concourse import dve_ops, mybir
    from concourse.masks import make_identity

    f32 = mybir.dt.float32
    bf16 = mybir.dt.bfloat16
    i16 = mybir.dt.int16
    Alu = mybir.AluOpType
    Act = mybir.ActivationFunctionType

    nc = bacc.Bacc("TRN2", debug=False)
    zseg_in = nc.dram_tensor("zt_seg", [D, SEG], bf16, kind="ExternalInput").ap()
    zpos_in = nc.dram_tensor("zt_pos", [D, PPAD], bf16, kind="ExternalInput").ap()
    mneg_in = nc.dram_tensor("mneg", [D, PPAD], bf16, kind="ExternalInput").ap()
    maskp_in = nc.dram_tensor("maskp", [D, SEGP], bf16, kind="ExternalInput").ap()
    sums_out = nc.dram_tensor("sums", [128, 2 * NT], f32, kind="ExternalOutput").ap()

    with tile.TileContext(nc) as tc:
        with (
            tc.tile_pool(name="const", bufs=1) as const,
            tc.tile_pool(name="zsegp", bufs=2) as zsegp,
            tc.tile_pool(name="zposp", bufs=2) as zposp,
            tc.tile_pool(name="mnegp", bufs=1) as mnegp,
            tc.tile_pool(name="maskpp", bufs=2) as maskpp,
            tc.tile_pool(name="ps", bufs=4, space="PSUM") as psp,
            tc.tile_pool(name="esb", bufs=3) as ep,
            tc.tile_pool(name="escrp", bufs=4) as escrp,
            tc.tile_pool(name="accp", bufs=2) as accp,
        ):
            npos = nt * 128
            # mneg is tiny and constant: load it once via the ACT HW queue,
            # before the exp-table load occupies the ACT sequencer
            mneg = mnegp.tile([D, PPAD], bf16)
            nc.scalar.dma_start(out=mneg[:, 0:npos], in_=mneg_in[:, 0:npos])
            # warm the ACT exp table while the first DMAs are in flight
            warm = const.tile([128, 1], f32)
            nc.vector.memset(warm, 0.0)
            nc.scalar.activation(out=warm, in_=warm, func=Act.Exp)
            ident = const.tile([128, 128], bf16)
            make_identity(nc, ident)

            def body():
                zseg = zsegp.tile([D, SEG], bf16, tag="zseg")
                zpos = zposp.tile([D, PPAD], bf16, tag="zpos")
                maskp = maskpp.tile([D, SEGP], bf16, tag="maskp")
                # SP HW queue feeds the first-use operands in order; the
                # gpsimd SWDGE queue carries the later stationaries.
                nc.sync.dma_start(out=zpos[:, 0:128], in_=zpos_in[:, 0:128])
                nc.sync.dma_start(out=zseg[:, 0:512], in_=zseg_in[:, 0:512])
                nc.sync.dma_start(out=zseg[:, 512:1024], in_=zseg_in[:, 512:1024])
                nc.gpsimd.dma_start(out=zpos[:, 128:npos], in_=zpos_in[:, 128:npos])
                nc.gpsimd.dma_start(out=maskp, in_=maskp_in)

                sums_sb = accp.tile([128, 2 * NT], f32, tag="sums_sb")
                if nt < NT:
                    nc.gpsimd.memset(sums_sb, 0.0)
                for t in range(nt):
                    sim = psp.tile([128, SEG], f32, tag="sim")
                    nc.tensor.matmul(
                        sim[:, 0:512],
                        lhsT=zpos[:, t * 128 : (t + 1) * 128],
                        rhs=zseg[:, 0:512],
                        start=True,
                        stop=False,
                    )
                    nc.tensor.matmul(
                        sim[:, 512:1024],
                        lhsT=zpos[:, t * 128 : (t + 1) * 128],
                        rhs=zseg[:, 512:1024],
                        start=True,
                        stop=True,
                    )
                    # self-similarity kill: PE accumulates -1e30 (via the
                    # identity stationary) onto the pos-first 128-col region
                    nc.tensor.matmul(
                        sim[:, 0:SEGP],
                        lhsT=ident,
                        rhs=mneg[:, t * SEGP : (t + 1) * SEGP],
                        start=False,
                        stop=True,
                    )
                    if t in ACT_TILES:
                        E = ep.tile([128, SEG], bf16, tag="E")
                        nc.scalar.activation(
                            out=E,
                            in_=sim,
                            func=Act.Exp,
                            scale=float(1.0 / TEMP),
                            accum_out=sums_sb[:, 2 * t : 2 * t + 1],
                        )
                        Ebf = E
                    else:
                        E16 = ep.tile([128, SEG], i16, tag="E")
                        nc.vector.tensor_scalar(
                            out=E16,
                            in0=sim,
                            scalar1=float(SCHRA_A),
                            scalar2=float(SCHRA_B),
                            op0=Alu.mult,
                            op1=Alu.add,
                        )
                        Ebf = E16.bitcast(bf16)
                        # single-src tensor_scalar runs in 2x/4x DVE mode
                        escr2 = ep.tile([128, SEG], bf16, tag="escr2")
                        nc.vector.tensor_scalar(
                            out=escr2,
                            in0=Ebf,
                            scalar1=1.0,
                            scalar2=0.0,
                            op0=Alu.mult,
                            op1=Alu.add,
                            accum_out=sums_sb[:, 2 * t : 2 * t + 1],
                        )
                    # masked pos-column sum.  The native InstTensorTensorReduce
                    # faults on HW; the last tile (exit critical path) uses the
                    # HW-validated custom-DVE op (accum = s0 + sum in0*in1*s1),
                    # earlier tiles run on the otherwise-idle GPSIMD engine.
                    escr = escrp.tile([128, SEGP], bf16, tag="escr")
                    if t == nt - 1:
                        nc.vector._custom_dve(
                            dve_ops.TENSOR_TENSOR_REDUCE,
                            out=escr,
                            in0=Ebf[:, 0:SEGP],
                            in1=maskp,
                            s0=0.0,
                            s1=1.0,
                            accum_out=sums_sb[:, 2 * t + 1 : 2 * t + 2],
                        )
                    else:
                        # masked multiply on the idle GPSIMD engine; the tiny
                        # 4x-mode tensor_scalar accumulate stays on the DVE
                        nc.gpsimd.tensor_mul(out=escr, in0=Ebf[:, 0:SEGP], in1=maskp)
                        escr3 = escrp.tile([128, SEGP], bf16, tag="escr3")
                        nc.vector.tensor_scalar(
                            out=escr3,
                            in0=escr,
                            scalar1=1.0,
                            scalar2=0.0,
                            op0=Alu.mult,
                            op1=Alu.add,
                            accum_out=sums_sb[:, 2 * t + 1 : 2 * t + 2],
                        )
                    if t == nt - 2:
                        # ship finished tiles early to shorten the exit tail
                        nc.sync.dma_start(
                            out=sums_out[:, 0 : 2 * (nt - 1)],
                            in_=sums_sb[:, 0 : 2 * (nt - 1)],
                        )
                nc.sync.dma_start(
                    out=sums_out[:, 2 * (nt - 1) : 2 * NT],
                    in_=sums_sb[:, 2 * (nt - 1) : 2 * NT],
                )

            for _rep in range(repeat):
                body()

    nc.compile()
    return nc


def _get_nc(repeat: int = 1, nt: int = 5):
    key = ("nc", repeat, nt)
    if key not in _CACHE:
        _CACHE[key] = _build(repeat, nt)
    return _CACHE[key]


def _host_prep(z_flowed: np.ndarray, attributes: np.ndarray):
    """Returns (in_maps, meta) or None if the data needs the host fallback."""
    import ml_dtypes

    z = np.asarray(z_flowed, dtype=np.float32)
    attrs = np.asarray(attributes, dtype=np.float32)
    p = attrs.sum(axis=1) == float(A)
    posidx = np.nonzero(p)[0]
    P = int(posidx.size)
    if P < 2 or P > PPAD:
        return None

    norm = np.maximum(np.sqrt((z.astype(np.float64) ** 2).sum(axis=1)), EPS)
    zn = (z / norm[:, None].astype(np.float32)).astype(ml_dtypes.bfloat16)

    zt_pos = np.zeros((PPAD, D), dtype=ml_dtypes.bfloat16)
    zt_pos[:P] = zn[posidx]
    zt_posT = np.ascontiguousarray(zt_pos.T)

    gpos = {int(i): g for g, i in enumerate(posidx)}  # row -> global pos idx

    in_maps = []
    for c in range(NCORES):
        lo, hi = c * SEG, (c + 1) * SEG
        segpos = posidx[(posidx >= lo) & (posidx < hi)]
        m_c = int(segpos.size)
        if m_c > SEGP:
            return None
        nonpos = np.setdiff1d(np.arange(lo, hi), segpos)
        order = np.concatenate([segpos, nonpos])
        zt_seg = np.ascontiguousarray(zn[order].T)

        mneg = np.zeros((D, PPAD), dtype=ml_dtypes.bfloat16)
        for k, i in enumerate(segpos):
            g = gpos[int(i)]
            mneg[g % 128, (g // 128) * SEGP + k] = -1e30
        maskp = np.zeros((D, SEGP), dtype=ml_dtypes.bfloat16)
        maskp[:, :m_c] = 1.0

        in_maps.append(
            {
                "zt_seg": zt_seg,
                "zt_pos": zt_posT,
                "mneg": mneg,
                "maskp": maskp,
            }
        )
    return in_maps, (P, posidx)


def make_in_maps(z_flowed: np.ndarray, attributes: np.ndarray):
    prep = _host_prep(z_flowed, attributes)
    assert prep is not None
    return prep[0]


def finish_host(results, attributes):
    """results: list of per-core dicts with 'sums' [128, 2*NT] f32."""
    attrs = np.asarray(attributes, dtype=np.float32)
    p = attrs.sum(axis=1) == float(A)
    P = int(p.sum())
    all_sum = np.zeros(PPAD, np.float64)
    pos_raw = np.zeros(PPAD, np.float64)
    for c in range(NCORES):
        s = np.asarray(results[c]["sums"], dtype=np.float64)
        for t in range(NT):
            all_sum[t * 128 : (t + 1) * 128] += s[:, 2 * t]
            pos_raw[t * 128 : (t + 1) * 128] += s[:, 2 * t + 1]
    all_sum = all_sum[:P]
    pos_sum = pos_raw[:P] + float(B - P + 1)
    loss_i = np.log(all_sum) - np.log(np.maximum(pos_sum, EPS))
    return np.float32(loss_i.mean())


def _host_fallback(z_flowed, attributes):
    z = np.asarray(z_flowed, dtype=np.float64)
    attrs = np.asarray(attributes, dtype=np.float64)
    Bn = z.shape[0]
    norm = np.maximum(np.linalg.norm(z, axis=1, keepdims=True), EPS)
    zn = z / norm
    sim = (zn @ zn.T) / TEMP
    asim = attrs @ attrs.T
    mask = (asim == attrs.shape[1]).astype(np.float64)
    np.fill_diagonal(mask, 0.0)
    num_pos = mask.sum(axis=1)
    pos_sum = np.exp(sim * mask).sum(axis=1)
    all_exp = np.exp(sim)
    all_sum = all_exp.sum(axis=1) - np.diagonal(all_exp)
    loss_i = np.log(all_sum) - np.log(np.maximum(pos_sum, EPS))
    valid = (num_pos > 0) & (all_sum > 0) & (pos_sum > 0)
    cnt = int(valid.sum())
    total = float(np.where(valid, loss_i, 0.0).sum())
    loss = total / max(cnt, 1) if cnt > 0 else 0.0
    return np.float32(loss)


def kernel(z_flowed: np.ndarray, attributes: np.ndarray) -> np.ndarray:
    from concourse.bass_utils import run_bass_kernel_spmd

    prep = _host_prep(z_flowed, attributes)
    if prep is None:
        return _host_fallback(z_flowed, attributes)
    in_maps, (P, _) = prep

    nt = max(1, min(NT, -(-P // 128)))
    nc = _get_nc(nt=nt)
    res = run_bass_kernel_spmd(nc, in_maps, list(range(NCORES)))
    _CACHE["last_result"] = res
    return finish_host(res.results, attributes)
